# revision 56
# baseline (speedup 1.0000x reference)
"""Trainium2 Bass kernel for nn_NetworkODEModel (gnn_message_passing).

Device computes the O(N^2) pair-coupling; everything O(N) rides the DMA:
  host:  v' = x@cW1b + cb1  (vT dup layout),  u = x@cW1a  (uT dup layout),
         node_out = MLP_n(x),  A = sigmoid(A_p - I/eps)*(1-I),
         accB = node_out + cbo*rowsum(A)
  device, per 2-pair group (128 groups/core, i sharded across 8 cores):
    ACT : c1 = Prelu(vT_slice + u_col)          (128,256) bf16, alpha=.01
    PE  : psz = cW2^T @ c1                      2 quadrant matmuls, W2 resident
    DVE : s_col = sum_j leaky(psz + cb2)*A_row  ONE fused custom op
  epilogue: out = s @ cWo + accB

The custom DVE op LEAKY_BIAS_MUL_REDUCE (registered at import into the
per-NEFF uop table):
  out   = max(in0+s0, (in0+s0)*imm2) * in1
  accum = sum(out)
"""

import os
import numpy as np

import concourse.bass as bass
import concourse.mybir as mybir
import concourse.tile as tile
from concourse import bacc
from concourse.bass_utils import run_bass_kernel_spmd

F32 = mybir.dt.float32
BF16 = mybir.dt.bfloat16
AOP = mybir.AluOpType
ACTF = mybir.ActivationFunctionType

B, N, D, HN, HC = 8, 256, 16, 64, 64
EPS = 1e-5
NCORES = 8
IPC = N // NCORES          # 32 i-rows per core
NPAIR = B * IPC            # 256 (b,i) pairs per core
NGROUP = NPAIR // 2        # 128 two-pair groups

# ---------------------------------------------------------------------------
# Custom DVE op: out = leaky(in0 + s0) * in1 ; accum_out = sum_j out
# ---------------------------------------------------------------------------
import concourse.dve_ops as dve_ops
from concourse.dve_spec import (
    AluOp,
    C0,
    C1,
    C2,
    Spec,
    Src0,
    Src1,
    Zero,
    _has_src1,
    lower,
    maxx,
)
from concourse.dve_spec import scan as dve_scan
from concourse.dve_uop import DveOpSpec
from operator import add as _add


def _register_op(name, spec, subdim=False):
    for o in dve_ops.OPS:
        if o.name == name:
            return o
    if name not in dve_ops._SUB_OPCODE_FOR_NAME:
        row = max(dve_ops._SUB_OPCODE_FOR_NAME.values()) + 1
        assert row < 0x20, row
        dve_ops._SUB_OPCODE_FOR_NAME[name] = row
    shas = {}
    for ver in ("v3", "v4"):
        s = DveOpSpec(
            name=name,
            opcode=dve_ops.get_dve_sub_opcode(name),
            uops=lower(spec, ver=ver),
            rd1_en=_has_src1(spec),
        )
        shas[ver] = s.sha(ver)
    op = dve_ops.DveOp(name, spec, subdim, shas)
    dve_ops.OPS.append(op)
    return op


def _leaky_ref(in0, in1, c0, c1, c2):
    x = in0.astype(np.float32) + c0
    b = (np.maximum(x, x * c2) * in1).astype(np.float32)
    return b, b.reshape(b.shape[0], -1).sum(axis=-1, keepdims=True)


_x = Src0 + C0
LEAKY_OP = _register_op(
    "LEAKY_BIAS_MUL_REDUCE",
    Spec(body=maxx(_x, _x * C2) * Src1, accum=_add, accum_init=Zero,
         reference=_leaky_ref),
)


def _leaky_scan_ref(in0, in1, c0, c1, c2):
    x = in0.astype(np.float32) + c0
    b = np.maximum(x, x * c1) * in1.reshape(in1.shape[0], -1)
    return np.cumsum(b, axis=-1).astype(np.float32)


# out = cumsum_j( leaky(in0 + s0) * in1 );  in1 broadcast [P, S, N]
LEAKY_SCAN_OP = _register_op(
    "LEAKY_MUL_SCAN",
    Spec(body=dve_scan(AluOp.ADD, maxx(_x, _x * C1) * Src1),
         reference=_leaky_scan_ref),
)


def _build_program(loop_k: int = 0):
    nc = bacc.Bacc(
        "TRN2",
        target_bir_lowering=False,
        debug=False,
        enable_asserts=False,
        num_devices=1,
    )

    def din(name, shape, dtype=F32):
        return nc.dram_tensor(name, list(shape), dtype, kind="ExternalInput")

    d_vT = din("vT", (128, B * N), BF16)    # host v' dup layout
    d_vTn = din("vTn", (128, N), BF16)      # host -v' dup, b=6 block only
    # misc pack: col 0 = cb2 stacked; cols 1:17 = cWo stacked;
    #            17:145 = u bias cols; 145:273 = -u bias cols
    d_misc = din("misc", (128, 1 + D + 2 * NGROUP))
    d_Abc = din("Abc", (128, IPC * N), BF16)  # A rows bcast (per core)
    d_accB = din("accB", (D, NPAIR))        # node_out + cbo*rowsumA (per core)
    d_W2 = din("W2", (128, 2 * HC), BF16)   # cols 0:64 cW2 x2; 64:128 -0.01*cW2 x2
    d_out = nc.dram_tensor("out_my", [D, NPAIR], F32, kind="ExternalOutput")

    # groups handled by the DVE relu-split path instead of the ACT Prelu
    # (always bp==3 so only the b=6 column block of -v' is ever needed);
    # ~22/128 groups balances ACT vs DVE
    def _is_ii(g):
        return g % 4 == 3 and (g // 4) % 5 != 2 and g >= 7

    with tile.TileContext(nc) as tc:
        with (
            tc.tile_pool(name="const", bufs=1) as cp,
            tc.tile_pool(name="abc", bufs=1) as ap_,
        ):
            # --- sync queue: misc + W2 first (first c1 / first matmul need
            #     them), then vT slices (+ small -v' block)
            # dummy activation so the Prelu table set loads before the DMAs land
            dummy = cp.tile([128, 2], BF16, tag="dummy")
            nc.vector.memset(dummy[:, :], 0.0)
            nc.scalar.activation(dummy[:, 0:1], dummy[:, 1:2], ACTF.Prelu,
                                 alpha=0.01)

            smisc = cp.tile([128, 1 + D + 2 * NGROUP], F32, tag="misc")
            nc.sync.dma_start(out=smisc[:, :], in_=d_misc.ap())
            scb2d = smisc[:, 0:1]
            scWo1 = smisc[:, 1 : 1 + D]
            suT = smisc[:, 1 + D : 1 + D + NGROUP]
            sunT = smisc[:, 1 + D + NGROUP :]
            svT = cp.tile([128, B * N], BF16, tag="vT")
            sW2 = cp.tile([128, 2 * HC], BF16, tag="W2")
            nc.sync.dma_start(out=svT[:, 0:1024], in_=d_vT.ap()[:, 0:1024])
            nc.sync.dma_start(out=sW2[:, :], in_=d_W2.ap())
            nc.sync.dma_start(out=svT[:, 1024:2048], in_=d_vT.ap()[:, 1024:2048])
            # bulk side data rides the software DGE on the (idle) Pool engine;
            # small/fast transfers first so they don't crowd the DMA engines
            svTn = cp.tile([128, N], BF16, tag="vTn")
            nc.gpsimd.dma_start(out=svTn[:, :], in_=d_vTn.ap())
            sAbc = ap_.tile([128, IPC * N], BF16)
            for c in range(0, 2048, 1024):
                nc.gpsimd.dma_start(
                    out=sAbc[:, c : c + 1024], in_=d_Abc.ap()[:, c : c + 1024]
                )
            saccB = cp.tile([D, NPAIR], F32, tag="accB")
            nc.gpsimd.dma_start(out=saccB[:, :], in_=d_accB.ap())
            for c in range(2048, IPC * N, 1024):
                nc.gpsimd.dma_start(
                    out=sAbc[:, c : c + 1024], in_=d_Abc.ap()[:, c : c + 1024]
                )

            # ---- main loop: 16 pairs of il-blocks (8 groups each).  Per pair:
            #   8x ACT c1 Prelu, 16 PE matmuls into one 4-bank psum tile
            #   (column slot of group (il,bp) = bp*512 + (il%2)*256, matching
            #    the A stream [A_il | A_il+1] repeated 4x),
            #   1 DVE scan op (cumsum of leaky(psz+cb2)*A over 2048 cols),
            #   1 extract of the 8 slot-cumulative columns.
            # s_cum blocks of 9 (col 9*ip = 0) -> one diff op recovers s_mat.
            s_mat = cp.tile([128, NGROUP], F32)
            s_cum = cp.tile([128, 9 * (IPC // 2)], F32)
            nc.gpsimd.memset(s_cum[:, :], 0.0)
            import contextlib

            with (
                tc.tile_pool(name="c1p", bufs=14) as rp,
                tc.tile_pool(name="rtp", bufs=6) as rtp,
                tc.tile_pool(name="scr", bufs=3) as sp,
                tc.tile_pool(name="pz", bufs=2, space="PSUM") as pzp,
                tc.For_i(0, loop_k, 1) if loop_k > 0 else contextlib.nullcontext(),
            ):
                ii_tiles = {}

                def _emit_ii_ts(gq):
                    rt = rtp.tile([128, N], BF16, tag="rt")
                    nc.vector.tensor_scalar(
                        out=rt[:, :], in0=svT[:, 6 * N : 7 * N],
                        scalar1=suT[:, gq : gq + 1], scalar2=0.0,
                        op0=AOP.add, op1=AOP.max,
                    )
                    rmt = rtp.tile([128, N], BF16, tag="rt")
                    nc.vector.tensor_scalar(
                        out=rmt[:, :], in0=svTn[:, :],
                        scalar1=sunT[:, gq : gq + 1], scalar2=0.0,
                        op0=AOP.add, op1=AOP.max,
                    )
                    ii_tiles[gq] = (rt, rmt)

                # pre-loop prefetch for the first pair of blocks
                for gq in range(8):
                    if _is_ii(gq):
                        _emit_ii_ts(gq)
                NP2 = IPC // 2
                for ip in range(NP2):
                    # prefetch the relu-split pairs one block-pair ahead
                    for gq in range((ip + 1) * 8, (ip + 2) * 8):
                        if gq < NGROUP and _is_ii(gq):
                            _emit_ii_ts(gq)
                    psz = pzp.tile([128, 8 * N], F32, tag="psz")
                    for ilo in range(2):
                        for bp in range(4):
                            g = (2 * ip + ilo) * 4 + bp
                            b0 = 2 * bp
                            cs = bp * 2 * N + ilo * N  # column slot in psz
                            if _is_ii(g):
                                rt, rmt = ii_tiles.pop(g)
                                for lo, tp in ((0, (0, 0)), (64, (64, 64))):
                                    nc.tensor.matmul(
                                        psz[lo : lo + 64, cs : cs + N],
                                        sW2[lo : lo + 64, 0:HC],
                                        rt[lo : lo + 64, :],
                                        start=True, stop=False, tile_position=tp,
                                    )
                                    nc.tensor.matmul(
                                        psz[lo : lo + 64, cs : cs + N],
                                        sW2[lo : lo + 64, HC:],
                                        rmt[lo : lo + 64, :],
                                        start=False, stop=True, tile_position=tp,
                                    )
                                continue
                            c1 = rp.tile([128, N], BF16, tag="c1")
                            nc.scalar.activation(
                                c1[:, :],
                                svT[:, b0 * N : (b0 + 1) * N],
                                ACTF.Prelu,
                                bias=suT[:, g : g + 1],
                                alpha=0.01,
                            )
                            nc.tensor.matmul(
                                psz[0:64, cs : cs + N], sW2[0:64, 0:HC],
                                c1[0:64, :],
                                start=True, stop=True, tile_position=(0, 0),
                            )
                            nc.tensor.matmul(
                                psz[64:128, cs : cs + N], sW2[64:128, 0:HC],
                                c1[64:128, :],
                                start=True, stop=True, tile_position=(64, 64),
                            )
                    zout = sp.tile([128, 8 * N], F32, tag="zout")
                    nc.vector._custom_dve(
                        LEAKY_SCAN_OP,
                        out=zout[:, :],
                        in0=psz[:, :],
                        in1=sAbc[:, ip * 2 * N : (ip + 1) * 2 * N]
                        .rearrange("p (x n) -> p x n", x=1)
                        .broadcast_to((128, 4, 2 * N)),
                        s0=scb2d,
                        s1=0.01,
                    )
                    # extract slot-cumulative cols: software DGE for the bulk,
                    # DVE for the last pair (keeps the epilogue dep short)
                    if ip < NP2 - 2:
                        nc.sync.dma_start(
                            out=s_cum[:, 9 * ip + 1 : 9 * ip + 9],
                            in_=zout[:, N - 1 : 8 * N : N],
                        )
                    else:
                        nc.vector.tensor_copy(
                            out=s_cum[:, 9 * ip + 1 : 9 * ip + 9],
                            in_=zout[:, N - 1 : 8 * N : N],
                        )
            # ---- epilogue in two halves: diff -> s@cWo -> +accB -> DMA out.
            #      Half 1 only depends on block-pairs 0..7, so it overlaps the
            #      tail of the main loop.
            # slot s = 2*bp + ilo;  group g = 8*ip + 4*ilo + bp
            smv = s_mat[:, :].rearrange("p (i l q) -> p i q l", i=IPC // 2, l=2, q=4)
            scv = s_cum[:, :].rearrange("p (i r) -> p i r", i=IPC // 2, r=9)
            with tc.tile_pool(name="pep", bufs=2, space="PSUM") as pe:
                psc1 = pe.tile([16, NGROUP], F32, tag="pc1")
                psc2 = pe.tile([16, NGROUP], F32, tag="pc2")
                final = cp.tile([D, NPAIR], F32)
                # dest col n = b*32+il; group col g = il*4+bp; top: b=2bp, bot: b=2bp+1
                fv = final[:, :].rearrange("p (q h i) -> p q h i", q=4, h=2, i=32)
                av = saccB[:, :].rearrange("p (q h i) -> p q h i", q=4, h=2, i=32)
                dov = d_out.ap().rearrange("p (q h i) -> p q h i", q=4, h=2, i=32)
                c1v = psc1[:, :].rearrange("p (i q) -> p q i", i=32, q=4)
                c2v = psc2[:, :].rearrange("p (i q) -> p q i", i=32, q=4)
                for lo, hi in ((0, NGROUP // 2), (NGROUP // 2, NGROUP)):
                    il_lo, il_hi = lo // 4, hi // 4
                    ip_lo, ip_hi = lo // 8, hi // 8
                    nc.vector.tensor_tensor(
                        out=smv[:, ip_lo:ip_hi, :, :],
                        in0=scv[:, ip_lo:ip_hi, 1:9].rearrange(
                            "p i (q l) -> p i q l", q=4, l=2
                        ),
                        in1=scv[:, ip_lo:ip_hi, 0:8].rearrange(
                            "p i (q l) -> p i q l", q=4, l=2
                        ),
                        op=AOP.subtract,
                    )
                    nc.tensor.matmul(
                        psc1[:, lo:hi], scWo1[0:64, :], s_mat[0:64, lo:hi],
                        start=True, stop=True, tile_position=(0, 0),
                    )
                    nc.tensor.matmul(
                        psc2[:, lo:hi], scWo1[64:128, :], s_mat[64:128, lo:hi],
                        start=True, stop=True, tile_position=(64, 0),
                    )
                    nc.vector.tensor_add(
                        out=fv[:, :, 0, il_lo:il_hi],
                        in0=c1v[:, :, il_lo:il_hi],
                        in1=av[:, :, 0, il_lo:il_hi],
                    )
                    nc.vector.tensor_add(
                        out=fv[:, :, 1, il_lo:il_hi],
                        in0=c2v[:, :, il_lo:il_hi],
                        in1=av[:, :, 1, il_lo:il_hi],
                    )
                    nc.sync.dma_start(
                        out=dov[:, :, :, il_lo:il_hi],
                        in_=fv[:, :, :, il_lo:il_hi],
                    )

    nc.compile()
    return nc


_NC_CACHE = {}


def _get_program():
    loop_k = int(os.environ.get("KERNEL_LOOP", "0"))
    key = ("nc", loop_k, os.environ.get("KERNEL_VARIANT", ""))
    if key not in _NC_CACHE:
        _NC_CACHE[key] = _build_program(loop_k)
    return _NC_CACHE[key]


def _np_leaky(v):
    return np.where(v > 0, v, np.float32(0.01) * v)


def _prep_in_maps(x, A_p, nW1, nb1, nW2, nb2, nWo, nbo, cW1, cb1, cW2, cb2, cWo, cbo):
    import ml_dtypes

    f = lambda a: np.ascontiguousarray(np.asarray(a, dtype=np.float32))
    x = f(x)
    A_p = f(A_p)
    nW1, nb1, nW2, nb2, nWo, nbo = f(nW1), f(nb1), f(nW2), f(nb2), f(nWo), f(nbo)
    cW1, cb1, cW2, cb2, cWo, cbo = f(cW1), f(cb1), f(cW2), f(cb2), f(cWo), f(cbo)

    # adjacency (stable sigmoid) with suppressed diagonal
    zmat = A_p - np.eye(N, dtype=np.float32) / EPS
    A = np.where(
        zmat >= 0,
        1.0 / (1.0 + np.exp(-np.clip(zmat, -80, 80))),
        np.exp(np.clip(zmat, -80, 80)) / (1.0 + np.exp(np.clip(zmat, -80, 80))),
    ).astype(np.float32)
    A *= 1.0 - np.eye(N, dtype=np.float32)
    rowsum = A.sum(axis=1)

    x2 = x.reshape(B * N, D)                      # b-major rows
    cW1a, cW1b = cW1[:D], cW1[D:]

    # host precompute: v' (dup layout), u, node MLP
    v = x2 @ cW1b + cb1                           # (2048, HC)
    vT = np.zeros((128, B * N), dtype=np.float32)
    vT[0:64] = v.T
    vT[64:128, 0 : (B - 1) * N] = v.T[:, N:]
    vTn = np.ascontiguousarray(-vT[:, 6 * N : 7 * N])  # b=6 block only
    u = x2 @ cW1a                                 # (2048, HC)
    h1 = _np_leaky(x2 @ nW1 + nb1)
    h2 = _np_leaky(h1 @ nW2 + nb2)
    nout = h2 @ nWo + nbo                         # (2048, D)

    stack2 = lambda m: np.ascontiguousarray(np.concatenate([m, m], axis=0))
    W2pack = np.concatenate([stack2(cW2), stack2(-0.01 * cW2)], axis=1)
    shared = {
        "vT": np.ascontiguousarray(vT.astype(ml_dtypes.bfloat16)),
        "vTn": np.ascontiguousarray(vTn.astype(ml_dtypes.bfloat16)),
        "W2": np.ascontiguousarray(W2pack.astype(ml_dtypes.bfloat16)),
    }
    cb2d = np.tile(cb2.reshape(HC, 1), (2, 1))
    cWo1 = stack2(cWo)

    in_maps = []
    cbo_f = cbo.reshape(D, 1)
    for k in range(NCORES):
        i0 = k * IPC
        # b-major column order: col = b*32 + il  ->  global row b*N + i0 + il
        cols = (np.arange(B)[:, None] * N + (i0 + np.arange(IPC))[None, :]).reshape(-1)
        # group-order bias columns: col g = il*4+bp -> top u(2bp, il), bot u(2bp+1, il)
        il_idx = np.arange(IPC).repeat(4)
        bp_idx = np.tile(np.arange(4), IPC)
        uT = np.empty((128, NGROUP), dtype=np.float32)
        uT[0:64] = u[(2 * bp_idx) * N + i0 + il_idx].T
        uT[64:128] = u[(2 * bp_idx + 1) * N + i0 + il_idx].T
        misc = np.concatenate([cb2d, cWo1, uT, -uT], axis=1).astype(np.float32)
        misc = np.ascontiguousarray(misc)
        accB = np.ascontiguousarray(
            nout[cols].T + cbo_f * np.tile(rowsum[i0 : i0 + IPC], B)[None, :]
        ).astype(np.float32)
        Achunk = A[i0 : i0 + IPC, :]              # (32, 256)
        Abc = np.ascontiguousarray(
            np.broadcast_to(
                Achunk.reshape(1, IPC * N).astype(ml_dtypes.bfloat16), (128, IPC * N)
            )
        )
        m = dict(shared)
        m["misc"] = misc
        m["accB"] = accB
        m["Abc"] = Abc
        in_maps.append(m)
    return in_maps


def kernel(**inputs) -> np.ndarray:
    nc = _get_program()
    in_maps = _prep_in_maps(**inputs)
    res = run_bass_kernel_spmd(nc, in_maps, core_ids=list(range(NCORES)))
    out = np.empty((B, N, D), dtype=np.float32)
    for k in range(NCORES):
        i0 = k * IPC
        om = res.results[k]["out_my"]  # (16, 256)
        out[:, i0 : i0 + IPC, :] = om.T.reshape(B, IPC, D)
    return out


# revision 57
# speedup vs baseline: 1.0092x; 1.0092x over previous
"""Trainium2 Bass kernel for nn_NetworkODEModel (gnn_message_passing).

Device computes the O(N^2) pair-coupling; everything O(N) rides the DMA:
  host:  v' = x@cW1b + cb1  (vT dup layout),  u = x@cW1a  (uT dup layout),
         node_out = MLP_n(x),  A = sigmoid(A_p - I/eps)*(1-I),
         accB = node_out + cbo*rowsum(A)
  device, per 2-pair group (128 groups/core, i sharded across 8 cores):
    ACT : c1 = Prelu(vT_slice + u_col)          (128,256) bf16, alpha=.01
    PE  : psz = cW2^T @ c1                      2 quadrant matmuls, W2 resident
    DVE : s_col = sum_j leaky(psz + cb2)*A_row  ONE fused custom op
  epilogue: out = s @ cWo + accB

The custom DVE op LEAKY_BIAS_MUL_REDUCE (registered at import into the
per-NEFF uop table):
  out   = max(in0+s0, (in0+s0)*imm2) * in1
  accum = sum(out)
"""

import os
import numpy as np

import concourse.bass as bass
import concourse.mybir as mybir
import concourse.tile as tile
from concourse import bacc
from concourse.bass_utils import run_bass_kernel_spmd

F32 = mybir.dt.float32
BF16 = mybir.dt.bfloat16
AOP = mybir.AluOpType
ACTF = mybir.ActivationFunctionType

B, N, D, HN, HC = 8, 256, 16, 64, 64
EPS = 1e-5
NCORES = 8
IPC = N // NCORES          # 32 i-rows per core
NPAIR = B * IPC            # 256 (b,i) pairs per core
NGROUP = NPAIR // 2        # 128 two-pair groups

# ---------------------------------------------------------------------------
# Custom DVE op: out = leaky(in0 + s0) * in1 ; accum_out = sum_j out
# ---------------------------------------------------------------------------
import concourse.dve_ops as dve_ops
from concourse.dve_spec import (
    AluOp,
    C0,
    C1,
    C2,
    Spec,
    Src0,
    Src1,
    Zero,
    _has_src1,
    lower,
    maxx,
)
from concourse.dve_spec import scan as dve_scan
from concourse.dve_uop import DveOpSpec
from operator import add as _add


def _register_op(name, spec, subdim=False):
    for o in dve_ops.OPS:
        if o.name == name:
            return o
    if name not in dve_ops._SUB_OPCODE_FOR_NAME:
        row = max(dve_ops._SUB_OPCODE_FOR_NAME.values()) + 1
        assert row < 0x20, row
        dve_ops._SUB_OPCODE_FOR_NAME[name] = row
    shas = {}
    for ver in ("v3", "v4"):
        s = DveOpSpec(
            name=name,
            opcode=dve_ops.get_dve_sub_opcode(name),
            uops=lower(spec, ver=ver),
            rd1_en=_has_src1(spec),
        )
        shas[ver] = s.sha(ver)
    op = dve_ops.DveOp(name, spec, subdim, shas)
    dve_ops.OPS.append(op)
    return op


def _leaky_ref(in0, in1, c0, c1, c2):
    x = in0.astype(np.float32) + c0
    b = (np.maximum(x, x * c2) * in1).astype(np.float32)
    return b, b.reshape(b.shape[0], -1).sum(axis=-1, keepdims=True)


_x = Src0 + C0
LEAKY_OP = _register_op(
    "LEAKY_BIAS_MUL_REDUCE",
    Spec(body=maxx(_x, _x * C2) * Src1, accum=_add, accum_init=Zero,
         reference=_leaky_ref),
)


def _leaky_scan_ref(in0, in1, c0, c1, c2):
    x = in0.astype(np.float32) + c0
    b = np.maximum(x, x * c1) * in1.reshape(in1.shape[0], -1)
    return np.cumsum(b, axis=-1).astype(np.float32)


# out = cumsum_j( leaky(in0 + s0) * in1 );  in1 broadcast [P, S, N]
LEAKY_SCAN_OP = _register_op(
    "LEAKY_MUL_SCAN",
    Spec(body=dve_scan(AluOp.ADD, maxx(_x, _x * C1) * Src1),
         reference=_leaky_scan_ref),
)


def _build_program(loop_k: int = 0):
    nc = bacc.Bacc(
        "TRN2",
        target_bir_lowering=False,
        debug=False,
        enable_asserts=False,
        num_devices=1,
    )

    def din(name, shape, dtype=F32):
        return nc.dram_tensor(name, list(shape), dtype, kind="ExternalInput")

    d_vT = din("vT", (128, B * N), BF16)    # host v' dup layout
    d_vTn = din("vTn", (128, N), BF16)      # host -v' dup, b=6 block only
    # misc pack: col 0 = cb2 stacked; cols 1:17 = cWo stacked;
    #            17:145 = u bias cols; 145:273 = -u bias cols
    d_misc = din("misc", (128, 1 + D + 2 * NGROUP))
    d_Abc = din("Abc", (128, IPC * N), BF16)  # A rows bcast (per core)
    d_accB = din("accB", (D, NPAIR))        # node_out + cbo*rowsumA (per core)
    d_W2 = din("W2", (128, 2 * HC), BF16)   # cols 0:64 cW2 x2; 64:128 -0.01*cW2 x2
    d_out = nc.dram_tensor("out_my", [D, NPAIR], F32, kind="ExternalOutput")

    # groups handled by the DVE relu-split path instead of the ACT Prelu
    # (always bp==3 so only the b=6 column block of -v' is ever needed);
    # ~22/128 groups balances ACT vs DVE
    def _is_ii(g):
        return g % 4 == 3 and (g // 4) % 4 != 2 and g >= 7

    with tile.TileContext(nc) as tc:
        with (
            tc.tile_pool(name="const", bufs=1) as cp,
            tc.tile_pool(name="abc", bufs=1) as ap_,
        ):
            # --- sync queue: misc + W2 first (first c1 / first matmul need
            #     them), then vT slices (+ small -v' block)
            # dummy activation so the Prelu table set loads before the DMAs land
            dummy = cp.tile([128, 2], BF16, tag="dummy")
            nc.vector.memset(dummy[:, :], 0.0)
            nc.scalar.activation(dummy[:, 0:1], dummy[:, 1:2], ACTF.Prelu,
                                 alpha=0.01)

            smisc = cp.tile([128, 1 + D + 2 * NGROUP], F32, tag="misc")
            nc.sync.dma_start(out=smisc[:, :], in_=d_misc.ap())
            scb2d = smisc[:, 0:1]
            scWo1 = smisc[:, 1 : 1 + D]
            suT = smisc[:, 1 + D : 1 + D + NGROUP]
            sunT = smisc[:, 1 + D + NGROUP :]
            svT = cp.tile([128, B * N], BF16, tag="vT")
            sW2 = cp.tile([128, 2 * HC], BF16, tag="W2")
            nc.sync.dma_start(out=svT[:, 0:1024], in_=d_vT.ap()[:, 0:1024])
            nc.sync.dma_start(out=sW2[:, :], in_=d_W2.ap())
            nc.sync.dma_start(out=svT[:, 1024:2048], in_=d_vT.ap()[:, 1024:2048])
            # bulk side data rides the software DGE on the (idle) Pool engine;
            # small/fast transfers first so they don't crowd the DMA engines
            svTn = cp.tile([128, N], BF16, tag="vTn")
            nc.gpsimd.dma_start(out=svTn[:, :], in_=d_vTn.ap())
            sAbc = ap_.tile([128, IPC * N], BF16)
            for c in range(0, 2048, 1024):
                nc.gpsimd.dma_start(
                    out=sAbc[:, c : c + 1024], in_=d_Abc.ap()[:, c : c + 1024]
                )
            saccB = cp.tile([D, NPAIR], F32, tag="accB")
            nc.gpsimd.dma_start(out=saccB[:, :], in_=d_accB.ap())
            for c in range(2048, IPC * N, 1024):
                nc.gpsimd.dma_start(
                    out=sAbc[:, c : c + 1024], in_=d_Abc.ap()[:, c : c + 1024]
                )

            # ---- main loop: 16 pairs of il-blocks (8 groups each).  Per pair:
            #   8x ACT c1 Prelu, 16 PE matmuls into one 4-bank psum tile
            #   (column slot of group (il,bp) = bp*512 + (il%2)*256, matching
            #    the A stream [A_il | A_il+1] repeated 4x),
            #   1 DVE scan op (cumsum of leaky(psz+cb2)*A over 2048 cols),
            #   1 extract of the 8 slot-cumulative columns.
            # s_cum blocks of 9 (col 9*ip = 0) -> one diff op recovers s_mat.
            s_mat = cp.tile([128, NGROUP], F32)
            s_cum = cp.tile([128, 9 * (IPC // 2)], F32)
            nc.gpsimd.memset(s_cum[:, :], 0.0)
            import contextlib

            with (
                tc.tile_pool(name="c1p", bufs=14) as rp,
                tc.tile_pool(name="rtp", bufs=6) as rtp,
                tc.tile_pool(name="scr", bufs=3) as sp,
                tc.tile_pool(name="pz", bufs=2, space="PSUM") as pzp,
                tc.For_i(0, loop_k, 1) if loop_k > 0 else contextlib.nullcontext(),
            ):
                ii_tiles = {}

                def _emit_ii_ts(gq):
                    rt = rtp.tile([128, N], BF16, tag="rt")
                    nc.vector.tensor_scalar(
                        out=rt[:, :], in0=svT[:, 6 * N : 7 * N],
                        scalar1=suT[:, gq : gq + 1], scalar2=0.0,
                        op0=AOP.add, op1=AOP.max,
                    )
                    rmt = rtp.tile([128, N], BF16, tag="rt")
                    nc.vector.tensor_scalar(
                        out=rmt[:, :], in0=svTn[:, :],
                        scalar1=sunT[:, gq : gq + 1], scalar2=0.0,
                        op0=AOP.add, op1=AOP.max,
                    )
                    ii_tiles[gq] = (rt, rmt)

                # pre-loop prefetch for the first pair of blocks
                for gq in range(8):
                    if _is_ii(gq):
                        _emit_ii_ts(gq)
                NP2 = IPC // 2
                for ip in range(NP2):
                    # prefetch the relu-split pairs one block-pair ahead
                    for gq in range((ip + 1) * 8, (ip + 2) * 8):
                        if gq < NGROUP and _is_ii(gq):
                            _emit_ii_ts(gq)
                    psz = pzp.tile([128, 8 * N], F32, tag="psz")
                    for ilo in range(2):
                        for bp in range(4):
                            g = (2 * ip + ilo) * 4 + bp
                            b0 = 2 * bp
                            cs = bp * 2 * N + ilo * N  # column slot in psz
                            if _is_ii(g):
                                rt, rmt = ii_tiles.pop(g)
                                for lo, tp in ((0, (0, 0)), (64, (64, 64))):
                                    nc.tensor.matmul(
                                        psz[lo : lo + 64, cs : cs + N],
                                        sW2[lo : lo + 64, 0:HC],
                                        rt[lo : lo + 64, :],
                                        start=True, stop=False, tile_position=tp,
                                    )
                                    nc.tensor.matmul(
                                        psz[lo : lo + 64, cs : cs + N],
                                        sW2[lo : lo + 64, HC:],
                                        rmt[lo : lo + 64, :],
                                        start=False, stop=True, tile_position=tp,
                                    )
                                continue
                            c1 = rp.tile([128, N], BF16, tag="c1")
                            nc.scalar.activation(
                                c1[:, :],
                                svT[:, b0 * N : (b0 + 1) * N],
                                ACTF.Prelu,
                                bias=suT[:, g : g + 1],
                                alpha=0.01,
                            )
                            nc.tensor.matmul(
                                psz[0:64, cs : cs + N], sW2[0:64, 0:HC],
                                c1[0:64, :],
                                start=True, stop=True, tile_position=(0, 0),
                            )
                            nc.tensor.matmul(
                                psz[64:128, cs : cs + N], sW2[64:128, 0:HC],
                                c1[64:128, :],
                                start=True, stop=True, tile_position=(64, 64),
                            )
                    zout = sp.tile([128, 8 * N], F32, tag="zout")
                    nc.vector._custom_dve(
                        LEAKY_SCAN_OP,
                        out=zout[:, :],
                        in0=psz[:, :],
                        in1=sAbc[:, ip * 2 * N : (ip + 1) * 2 * N]
                        .rearrange("p (x n) -> p x n", x=1)
                        .broadcast_to((128, 4, 2 * N)),
                        s0=scb2d,
                        s1=0.01,
                    )
                    # extract slot-cumulative cols: software DGE for the bulk,
                    # DVE for the last pair (keeps the epilogue dep short)
                    if ip < NP2 - 2:
                        nc.sync.dma_start(
                            out=s_cum[:, 9 * ip + 1 : 9 * ip + 9],
                            in_=zout[:, N - 1 : 8 * N : N],
                        )
                    else:
                        nc.vector.tensor_copy(
                            out=s_cum[:, 9 * ip + 1 : 9 * ip + 9],
                            in_=zout[:, N - 1 : 8 * N : N],
                        )
            # ---- epilogue in two halves: diff -> s@cWo -> +accB -> DMA out.
            #      Half 1 only depends on block-pairs 0..7, so it overlaps the
            #      tail of the main loop.
            # slot s = 2*bp + ilo;  group g = 8*ip + 4*ilo + bp
            smv = s_mat[:, :].rearrange("p (i l q) -> p i q l", i=IPC // 2, l=2, q=4)
            scv = s_cum[:, :].rearrange("p (i r) -> p i r", i=IPC // 2, r=9)
            with tc.tile_pool(name="pep", bufs=2, space="PSUM") as pe:
                psc1 = pe.tile([16, NGROUP], F32, tag="pc1")
                psc2 = pe.tile([16, NGROUP], F32, tag="pc2")
                final = cp.tile([D, NPAIR], F32)
                # dest col n = b*32+il; group col g = il*4+bp; top: b=2bp, bot: b=2bp+1
                fv = final[:, :].rearrange("p (q h i) -> p q h i", q=4, h=2, i=32)
                av = saccB[:, :].rearrange("p (q h i) -> p q h i", q=4, h=2, i=32)
                dov = d_out.ap().rearrange("p (q h i) -> p q h i", q=4, h=2, i=32)
                c1v = psc1[:, :].rearrange("p (i q) -> p q i", i=32, q=4)
                c2v = psc2[:, :].rearrange("p (i q) -> p q i", i=32, q=4)
                for lo, hi in ((0, NGROUP // 2), (NGROUP // 2, NGROUP)):
                    il_lo, il_hi = lo // 4, hi // 4
                    ip_lo, ip_hi = lo // 8, hi // 8
                    nc.vector.tensor_tensor(
                        out=smv[:, ip_lo:ip_hi, :, :],
                        in0=scv[:, ip_lo:ip_hi, 1:9].rearrange(
                            "p i (q l) -> p i q l", q=4, l=2
                        ),
                        in1=scv[:, ip_lo:ip_hi, 0:8].rearrange(
                            "p i (q l) -> p i q l", q=4, l=2
                        ),
                        op=AOP.subtract,
                    )
                    nc.tensor.matmul(
                        psc1[:, lo:hi], scWo1[0:64, :], s_mat[0:64, lo:hi],
                        start=True, stop=True, tile_position=(0, 0),
                    )
                    nc.tensor.matmul(
                        psc2[:, lo:hi], scWo1[64:128, :], s_mat[64:128, lo:hi],
                        start=True, stop=True, tile_position=(64, 0),
                    )
                    nc.vector.tensor_add(
                        out=fv[:, :, 0, il_lo:il_hi],
                        in0=c1v[:, :, il_lo:il_hi],
                        in1=av[:, :, 0, il_lo:il_hi],
                    )
                    nc.vector.tensor_add(
                        out=fv[:, :, 1, il_lo:il_hi],
                        in0=c2v[:, :, il_lo:il_hi],
                        in1=av[:, :, 1, il_lo:il_hi],
                    )
                    nc.sync.dma_start(
                        out=dov[:, :, :, il_lo:il_hi],
                        in_=fv[:, :, :, il_lo:il_hi],
                    )

    nc.compile()
    return nc


_NC_CACHE = {}


def _get_program():
    loop_k = int(os.environ.get("KERNEL_LOOP", "0"))
    key = ("nc", loop_k, os.environ.get("KERNEL_VARIANT", ""))
    if key not in _NC_CACHE:
        _NC_CACHE[key] = _build_program(loop_k)
    return _NC_CACHE[key]


def _np_leaky(v):
    return np.where(v > 0, v, np.float32(0.01) * v)


def _prep_in_maps(x, A_p, nW1, nb1, nW2, nb2, nWo, nbo, cW1, cb1, cW2, cb2, cWo, cbo):
    import ml_dtypes

    f = lambda a: np.ascontiguousarray(np.asarray(a, dtype=np.float32))
    x = f(x)
    A_p = f(A_p)
    nW1, nb1, nW2, nb2, nWo, nbo = f(nW1), f(nb1), f(nW2), f(nb2), f(nWo), f(nbo)
    cW1, cb1, cW2, cb2, cWo, cbo = f(cW1), f(cb1), f(cW2), f(cb2), f(cWo), f(cbo)

    # adjacency (stable sigmoid) with suppressed diagonal
    zmat = A_p - np.eye(N, dtype=np.float32) / EPS
    A = np.where(
        zmat >= 0,
        1.0 / (1.0 + np.exp(-np.clip(zmat, -80, 80))),
        np.exp(np.clip(zmat, -80, 80)) / (1.0 + np.exp(np.clip(zmat, -80, 80))),
    ).astype(np.float32)
    A *= 1.0 - np.eye(N, dtype=np.float32)
    rowsum = A.sum(axis=1)

    x2 = x.reshape(B * N, D)                      # b-major rows
    cW1a, cW1b = cW1[:D], cW1[D:]

    # host precompute: v' (dup layout), u, node MLP
    v = x2 @ cW1b + cb1                           # (2048, HC)
    vT = np.zeros((128, B * N), dtype=np.float32)
    vT[0:64] = v.T
    vT[64:128, 0 : (B - 1) * N] = v.T[:, N:]
    vTn = np.ascontiguousarray(-vT[:, 6 * N : 7 * N])  # b=6 block only
    u = x2 @ cW1a                                 # (2048, HC)
    h1 = _np_leaky(x2 @ nW1 + nb1)
    h2 = _np_leaky(h1 @ nW2 + nb2)
    nout = h2 @ nWo + nbo                         # (2048, D)

    stack2 = lambda m: np.ascontiguousarray(np.concatenate([m, m], axis=0))
    W2pack = np.concatenate([stack2(cW2), stack2(-0.01 * cW2)], axis=1)
    shared = {
        "vT": np.ascontiguousarray(vT.astype(ml_dtypes.bfloat16)),
        "vTn": np.ascontiguousarray(vTn.astype(ml_dtypes.bfloat16)),
        "W2": np.ascontiguousarray(W2pack.astype(ml_dtypes.bfloat16)),
    }
    cb2d = np.tile(cb2.reshape(HC, 1), (2, 1))
    cWo1 = stack2(cWo)

    in_maps = []
    cbo_f = cbo.reshape(D, 1)
    for k in range(NCORES):
        i0 = k * IPC
        # b-major column order: col = b*32 + il  ->  global row b*N + i0 + il
        cols = (np.arange(B)[:, None] * N + (i0 + np.arange(IPC))[None, :]).reshape(-1)
        # group-order bias columns: col g = il*4+bp -> top u(2bp, il), bot u(2bp+1, il)
        il_idx = np.arange(IPC).repeat(4)
        bp_idx = np.tile(np.arange(4), IPC)
        uT = np.empty((128, NGROUP), dtype=np.float32)
        uT[0:64] = u[(2 * bp_idx) * N + i0 + il_idx].T
        uT[64:128] = u[(2 * bp_idx + 1) * N + i0 + il_idx].T
        misc = np.concatenate([cb2d, cWo1, uT, -uT], axis=1).astype(np.float32)
        misc = np.ascontiguousarray(misc)
        accB = np.ascontiguousarray(
            nout[cols].T + cbo_f * np.tile(rowsum[i0 : i0 + IPC], B)[None, :]
        ).astype(np.float32)
        Achunk = A[i0 : i0 + IPC, :]              # (32, 256)
        Abc = np.ascontiguousarray(
            np.broadcast_to(
                Achunk.reshape(1, IPC * N).astype(ml_dtypes.bfloat16), (128, IPC * N)
            )
        )
        m = dict(shared)
        m["misc"] = misc
        m["accB"] = accB
        m["Abc"] = Abc
        in_maps.append(m)
    return in_maps


def kernel(**inputs) -> np.ndarray:
    nc = _get_program()
    in_maps = _prep_in_maps(**inputs)
    res = run_bass_kernel_spmd(nc, in_maps, core_ids=list(range(NCORES)))
    out = np.empty((B, N, D), dtype=np.float32)
    for k in range(NCORES):
        i0 = k * IPC
        om = res.results[k]["out_my"]  # (16, 256)
        out[:, i0 : i0 + IPC, :] = om.T.reshape(B, IPC, D)
    return out
